# revision 25
# baseline (speedup 1.0000x reference)
"""AdditiveAttention (d2l-style) Trainium2 kernel, SPMD over 8 NeuronCores.

Problem shapes: B=16, Q=64, K=1024, DQ=DK=256, H=128, DV=256.

Sharding: data-parallel over the QUERY axis (8 queries per core), with every
core processing all 16 batches. This keeps the single SPMD instruction
stream identical across cores while allowing the graph (built at runtime
from the actual valid_lens values) to skip fully-invalid key tiles — a
large win since valid_lens average ~K/2.

Per-core pipeline (batches processed smallest-first, software-pipelined):
  - host packs keys^T (valid 128-tiles only, partition-major so each batch
    is ONE contiguous DMA) and [values | ones] likewise, both fp16
  - PE: kproj^T = Wk^T @ keys^T per tile; qproj^T likewise
  - ACT: feat = tanh(kproj^T + qproj_col) with qproj as per-partition bias
  - PE: scores col = feat_tile^T @ wv into one [128k, T*8] PSUM tile
        (transposed scores; PE outputs must start at partition 0/32/64)
  - ACT: e = exp(scores) per group of batches (no max-subtract needed;
        |scores| <= sum|wv| ~ 9, exp stays in fp16 range)
  - PE: [out | denom] = e_task^T @ [V | 1] accumulated over valid tiles;
        invalid tail rows of the last tile are zeroed in V on the host
  - DVE: out = out * (1/denom) ; DMA out, all per-batch (no serial tail)
"""

import sys

sys.path.insert(0, "/opt/trn_rl_repo")

from contextlib import ExitStack

import numpy as np

import concourse.bass as bass  # noqa: F401  (bass must import before tile)
import concourse.tile as tile
from concourse import bacc, mybir
from concourse.bass_utils import run_bass_kernel_spmd

B, Q, KLEN, D, H, DV = 16, 64, 1024, 256, 128, 256
N_CORES = 8
QS = Q // N_CORES  # queries per core = 8
KT = 128  # key tile size
EXP_GROUP = 2  # batches per exp instruction

F16 = mybir.dt.float16
F32 = mybir.dt.float32


def _build_graph(order, tiles, vls):
    """order: batch processing order; tiles[b]: valid 128-key tiles; vls[b]: valid_lens."""
    ntiles = [int(tiles[b]) for b in order]
    nvalid = [int(vls[b]) for b in order]
    T = int(sum(ntiles))
    # offs[i] = first task index of the i-th processed batch
    offs = np.concatenate([[0], np.cumsum(ntiles)]).astype(int)

    nc = bacc.Bacc("TRN2", target_bir_lowering=False, debug=False, num_devices=N_CORES)
    # per-batch contiguous packs, partition-major: kT[i] is [128, nt_i*256]
    kT_d = nc.dram_tensor("kT", [KT, T * 2 * KT], F16, kind="ExternalInput").ap()
    v1_d = nc.dram_tensor("v1", [KT, T * (DV + 1)], F16, kind="ExternalInput").ap()
    qT_d = nc.dram_tensor("qT", [2, KT, B * QS], F16, kind="ExternalInput").ap()
    wq_d = nc.dram_tensor("wqc", [2, KT, H], F16, kind="ExternalInput").ap()
    wk_d = nc.dram_tensor("wkc", [2, KT, H], F16, kind="ExternalInput").ap()
    wv_d = nc.dram_tensor("wv", [H, 1], F16, kind="ExternalInput").ap()
    out_d = nc.dram_tensor("out", [B * QS, DV], F32, kind="ExternalOutput").ap()

    with tile.TileContext(nc) as tc, ExitStack() as ctx:
        const = ctx.enter_context(tc.tile_pool(name="const", bufs=1))
        spool = ctx.enter_context(tc.tile_pool(name="s_psum", bufs=1, space="PSUM"))
        epool = ctx.enter_context(tc.tile_pool(name="e_sb", bufs=2))
        vpool = ctx.enter_context(tc.tile_pool(name="vals", bufs=4))
        fin = ctx.enter_context(tc.tile_pool(name="fin", bufs=1))
        opool = ctx.enter_context(tc.tile_pool(name="o_psum", bufs=2, space="PSUM"))

        # constants
        wq_sb = [const.tile([KT, H], F16, tag=f"wq{c}", name=f"wq{c}") for c in range(2)]
        wk_sb = [const.tile([KT, H], F16, tag=f"wk{c}", name=f"wk{c}") for c in range(2)]
        qT_sb = [
            const.tile([KT, B * QS], F16, tag=f"qt{c}", name=f"qt{c}") for c in range(2)
        ]
        for c in range(2):
            nc.sync.dma_start(wq_sb[c][:], wq_d[c])
            nc.sync.dma_start(wk_sb[c][:], wk_d[c])
            nc.sync.dma_start(qT_sb[c][:], qT_d[c])
        wv_sb = const.tile([H, 1], F16, tag="wv", name="wv_sb")
        nc.sync.dma_start(wv_sb[:], wv_d[:])
        qproj_sb = const.tile([H, B * QS], F32, tag="qproj", name="qproj_sb")

        # transposed scores: partition = key-in-tile, free = task*QS + j
        S = spool.tile([KT, T * QS], F32)
        # partial-tile columns are only written up to their valid rows;
        # zero once so exp() of the stale rows is finite (killed by V=0)
        nc.vector.memset(S[:, :], 0.0)

        # ---- phase 0: query projection ----
        with tc.tile_pool(name="qp_psum", bufs=1, space="PSUM") as qpp:
            qp = qpp.tile([H, B * QS], F32)
            for c in range(2):
                nc.tensor.matmul(
                    qp[:], wq_sb[c][:], qT_sb[c][:], start=(c == 0), stop=(c == 1)
                )
            nc.vector.tensor_copy(qproj_sb[:], qp[:])

        vals_sb = {}  # processed-batch index -> [128, nt*(DV+1)] tile

        with (
            tc.tile_pool(name="kt_sb", bufs=4) as ktp,
            tc.tile_pool(name="kp_psum", bufs=4, space="PSUM") as kpp,
            tc.tile_pool(name="kproj_sb", bufs=3) as kjp,
            tc.tile_pool(name="prebias", bufs=3) as pbp,
            tc.tile_pool(name="feat", bufs=3) as fp,
        ):
            kprojs = {}
            prebiases = {}

            def emit_prebias(i):
                # prebias[:, j*vl2:(j+1)*vl2] = kproj + qproj[:, bq]  (DVE 4x fp16)
                vl_i = nvalid[i]
                vl2 = vl_i + (vl_i & 1)  # even segment width keeps 4B alignment
                kproj = kprojs.pop(i)
                pb = pbp.tile([H, QS * KLEN], F16, tag="pb", name="pb")
                for j in range(QS):
                    bq = order[i] * QS + j
                    nc.vector.tensor_scalar_add(
                        pb[:, j * vl2 : j * vl2 + vl_i],
                        kproj[:, :vl_i],
                        qproj_sb[:, bq : bq + 1],
                    )
                prebiases[i] = pb

            def emit_kproj(i):
                nt = ntiles[i]
                vlpad = nt * KT
                o = int(offs[i])
                # one contiguous DMA for the whole batch's [V|1], then keys^T
                vt = vpool.tile([KT, 8 * (DV + 1)], F16, tag="v1", name="vt")
                nc.sync.dma_start(
                    vt[:, : nt * (DV + 1)],
                    v1_d[:, o * (DV + 1) : (o + nt) * (DV + 1)],
                )
                vals_sb[i] = vt
                kt = ktp.tile([KT, 8 * 2 * KT], F16, tag="kt", name="kt")
                nc.sync.dma_start(
                    kt[:, : nt * 2 * KT], kT_d[:, o * 2 * KT : (o + nt) * 2 * KT]
                )
                vl_i = nvalid[i]
                kproj = kjp.tile([H, KLEN], F16, tag="kproj", name="kproj")
                for t0 in range(0, nt, 4):
                    t1 = min(t0 + 4, nt)
                    cw = min(vl_i, t1 * KT) - t0 * KT
                    kp = kpp.tile([H, 512], F32, tag="kp", name="kp")
                    for t in range(t0, t1):
                        w = min(KT, vl_i - t * KT)
                        sl = kp[:, (t - t0) * KT : (t - t0) * KT + w]
                        k0 = kt[:, (2 * t) * KT : (2 * t) * KT + w]
                        k1 = kt[:, (2 * t + 1) * KT : (2 * t + 1) * KT + w]
                        nc.tensor.matmul(sl, wk_sb[0][:], k0, start=True, stop=False)
                        nc.tensor.matmul(sl, wk_sb[1][:], k1, start=False, stop=True)
                    nc.vector.tensor_copy(
                        kproj[:, t0 * KT : t0 * KT + cw], kp[:, :cw]
                    )
                kprojs[i] = kproj

            def emit_exp_and_av(i0, i1):
                # exp over the column range of processed batches [i0, i1)
                c0, c1 = int(offs[i0]) * QS, int(offs[i1]) * QS
                e = epool.tile([KT, EXP_GROUP * 8 * QS], F16, tag="e", name="e", bufs=3)
                nc.scalar.activation(
                    e[:, : c1 - c0], S[:, c0:c1], mybir.ActivationFunctionType.Exp
                )
                for i in range(i0, i1):
                    b = order[i]
                    nt = ntiles[i]
                    Ob = opool.tile([QS, DV + 1], F32, tag="ob", name="Ob")
                    for t in range(nt):
                        ec = (int(offs[i]) + t) * QS - c0
                        nc.tensor.matmul(
                            Ob[:],
                            e[:, ec : ec + QS],
                            vals_sb[i][:, t * (DV + 1) : (t + 1) * (DV + 1)],
                            start=(t == 0),
                            stop=(t == nt - 1),
                        )
                    vals_sb[i] = None
                    recip = fin.tile([QS, 1], F32, tag="recip", name="recip", bufs=2)
                    nc.vector.reciprocal(recip[:], Ob[:, DV : DV + 1])
                    outf = fin.tile([QS, DV], F32, tag="outf", name="outf", bufs=3)
                    nc.vector.tensor_scalar_mul(outf[:], Ob[:, :DV], recip[:])
                    nc.sync.dma_start(out_d[b * QS : (b + 1) * QS, :], outf[:])

            # kproj two batches ahead, prebias one batch ahead, so PE/DVE
            # feed ACT's single wide tanh per batch without stalls.
            emit_kproj(0)
            emit_kproj(1)
            for i in range(B):
                if i + 1 < B:
                    emit_prebias(i + 1)
                if i + 2 < B:
                    emit_kproj(i + 2)
                nt = ntiles[i]
                vl_i = nvalid[i]
                vl2 = vl_i + (vl_i & 1)
                if i == 0:
                    # ramp: per-q biased tanh straight off kproj — shorter
                    # dependency chain than the prebias path
                    kproj0 = kprojs.pop(0)
                    feat = fp.tile([H, QS * KLEN], F16, tag="feat", name="feat")
                    for j in range(QS):
                        bq = order[0] * QS + j
                        nc.scalar.activation(
                            feat[:, j * vl2 : j * vl2 + vl_i],
                            kproj0[:, :vl_i],
                            mybir.ActivationFunctionType.Tanh,
                            bias=qproj_sb[:, bq : bq + 1],
                        )
                else:
                    pb = prebiases.pop(i)
                    feat = fp.tile([H, QS * KLEN], F16, tag="feat", name="feat")
                    nc.scalar.activation(
                        feat[:, : QS * vl2],
                        pb[:, : QS * vl2],
                        mybir.ActivationFunctionType.Tanh,
                    )
                for j in range(QS):
                    for t in range(nt):
                        w = min(KT, vl_i - t * KT)
                        gcol = (int(offs[i]) + t) * QS + j
                        nc.tensor.matmul(
                            S[:w, gcol : gcol + 1],
                            feat[:, j * vl2 + t * KT : j * vl2 + t * KT + w],
                            wv_sb[:],
                            start=True,
                            stop=True,
                        )
                # lag exp/AV by one group: by the time ACT reaches this exp
                # in its in-order stream, the group's matvecs are long done,
                # so exp doesn't head-of-line-block the next tanh.
                if i % EXP_GROUP == EXP_GROUP - 1 and i >= 2 * EXP_GROUP - 1:
                    emit_exp_and_av(i - 2 * EXP_GROUP + 1, i - EXP_GROUP + 1)
            emit_exp_and_av(B - EXP_GROUP, B)

    nc.compile()
    return nc


def kernel(queries, keys, values, valid_lens, Wq, Wk, wv):
    queries = np.asarray(queries, dtype=np.float32)
    keys = np.asarray(keys, dtype=np.float32)
    values = np.asarray(values, dtype=np.float32)
    vl = np.asarray(valid_lens).astype(np.int64)
    Wq = np.asarray(Wq, dtype=np.float32)
    Wk = np.asarray(Wk, dtype=np.float32)
    wv = np.asarray(wv, dtype=np.float32)

    tiles = np.maximum(1, -(-vl // KT))  # ceil, >=1
    asc = np.argsort(tiles, kind="stable")
    # four smallest batches first (fast ramp, light DMA warmup flood), then
    # descending so the last processed batches (the exp/AV tail) are small
    order = np.concatenate([asc[:4], asc[4:][::-1]])
    T = int(tiles.sum())

    # packed keys^T, partition-major: column block per task of [2*128] (d-chunks)
    kT_pack = np.empty((KT, T, 2, KT), dtype=np.float16)
    v1_pack = np.empty((KT, T, DV + 1), dtype=np.float16)
    v1_pack[:, :, DV] = 1.0
    i = 0
    for i_proc in range(B):
        b = int(order[i_proc])
        nt = int(tiles[b])
        for t in range(nt):
            ksl = keys[b, t * KT : (t + 1) * KT, :]  # [128 k, 256 d]
            # kT_pack[p, i, c, :] = keys[b, t*KT + (c*128..), p]^T chunks
            kT_pack[:, i, 0, :] = ksl[:, :KT].T
            kT_pack[:, i, 1, :] = ksl[:, KT:].T
            v1_pack[:, i, :DV] = values[b, t * KT : (t + 1) * KT, :]
            if t == nt - 1:
                rows = int(vl[b]) - t * KT
                v1_pack[rows:, i, :] = 0.0  # mask invalid keys via V and ones col
            i += 1
    kT_pack = kT_pack.reshape(KT, T * 2 * KT)
    v1_pack = v1_pack.reshape(KT, T * (DV + 1))

    wqc = Wq.reshape(2, KT, H).astype(np.float16)
    wkc = Wk.reshape(2, KT, H).astype(np.float16)
    wv_c = wv.reshape(H, 1).astype(np.float16)

    nc = _build_graph(order, tiles, vl)

    in_maps = []
    for c in range(N_CORES):
        qc = queries[:, c * QS : (c + 1) * QS, :].reshape(B * QS, D)
        qT = np.ascontiguousarray(qc.T).reshape(2, KT, B * QS).astype(np.float16)
        in_maps.append(
            {
                "kT": kT_pack,
                "v1": v1_pack,
                "qT": qT,
                "wqc": wqc,
                "wkc": wkc,
                "wv": wv_c,
            }
        )

    res = run_bass_kernel_spmd(nc, in_maps, core_ids=list(range(N_CORES)))

    out = np.empty((B, Q, DV), dtype=np.float32)
    for c in range(N_CORES):
        out[:, c * QS : (c + 1) * QS, :] = res.results[c]["out"].reshape(B, QS, DV)
    return out


# revision 26
# speedup vs baseline: 1.1429x; 1.1429x over previous
"""AdditiveAttention (d2l-style) Trainium2 kernel, SPMD over 8 NeuronCores.

Problem shapes: B=16, Q=64, K=1024, DQ=DK=256, H=128, DV=256.

Sharding: data-parallel over the QUERY axis (8 queries per core), with every
core processing all 16 batches. This keeps the single SPMD instruction
stream identical across cores while allowing the graph (built at runtime
from the actual valid_lens values) to skip fully-invalid key tiles — a
large win since valid_lens average ~K/2.

Per-core pipeline (batches processed smallest-first, software-pipelined):
  - host packs keys^T (valid 128-tiles only, partition-major so each batch
    is ONE contiguous DMA) and [values | ones] likewise, both fp16
  - PE: kproj^T = Wk^T @ keys^T per tile; qproj^T likewise
  - ACT: feat = tanh(kproj^T + qproj_col) with qproj as per-partition bias
  - PE: scores col = feat_tile^T @ wv into one [128k, T*8] PSUM tile
        (transposed scores; PE outputs must start at partition 0/32/64)
  - ACT: e = exp(scores) per group of batches (no max-subtract needed;
        |scores| <= sum|wv| ~ 9, exp stays in fp16 range)
  - PE: [out | denom] = e_task^T @ [V | 1] accumulated over valid tiles;
        invalid tail rows of the last tile are zeroed in V on the host
  - DVE: out = out * (1/denom) ; DMA out, all per-batch (no serial tail)
"""

import sys

sys.path.insert(0, "/opt/trn_rl_repo")

from contextlib import ExitStack

import numpy as np

import concourse.bass as bass  # noqa: F401  (bass must import before tile)
import concourse.tile as tile
from concourse import bacc, mybir
from concourse.bass_utils import run_bass_kernel_spmd

B, Q, KLEN, D, H, DV = 16, 64, 1024, 256, 128, 256
N_CORES = 8
QS = Q // N_CORES  # queries per core = 8
KT = 128  # key tile size
EXP_GROUP = 2  # batches per exp instruction

F16 = mybir.dt.float16
F32 = mybir.dt.float32


def _build_graph(order, tiles, vls):
    """order: batch processing order; tiles[b]: valid 128-key tiles; vls[b]: valid_lens."""
    ntiles = [int(tiles[b]) for b in order]
    nvalid = [int(vls[b]) for b in order]
    T = int(sum(ntiles))
    # offs[i] = first task index of the i-th processed batch
    offs = np.concatenate([[0], np.cumsum(ntiles)]).astype(int)

    nc = bacc.Bacc("TRN2", target_bir_lowering=False, debug=False, num_devices=N_CORES)
    # per-batch contiguous packs, partition-major: kT[i] is [128, nt_i*256]
    kT_d = nc.dram_tensor("kT", [KT, T * 2 * KT], F16, kind="ExternalInput").ap()
    v1_d = nc.dram_tensor("v1", [KT, T * (DV + 1)], F16, kind="ExternalInput").ap()
    qT_d = nc.dram_tensor("qT", [2, KT, B * QS], F16, kind="ExternalInput").ap()
    wq_d = nc.dram_tensor("wqc", [2, KT, H], F16, kind="ExternalInput").ap()
    wk_d = nc.dram_tensor("wkc", [2, KT, H], F16, kind="ExternalInput").ap()
    wv_d = nc.dram_tensor("wv", [H, 1], F16, kind="ExternalInput").ap()
    out_d = nc.dram_tensor("out", [B * QS, DV], F32, kind="ExternalOutput").ap()

    with tile.TileContext(nc) as tc, ExitStack() as ctx:
        const = ctx.enter_context(tc.tile_pool(name="const", bufs=1))
        spool = ctx.enter_context(tc.tile_pool(name="s_psum", bufs=1, space="PSUM"))
        epool = ctx.enter_context(tc.tile_pool(name="e_sb", bufs=2))
        vpool = ctx.enter_context(tc.tile_pool(name="vals", bufs=4))
        fin = ctx.enter_context(tc.tile_pool(name="fin", bufs=1))
        opool = ctx.enter_context(tc.tile_pool(name="o_psum", bufs=2, space="PSUM"))

        # constants
        wq_sb = [const.tile([KT, H], F16, tag=f"wq{c}", name=f"wq{c}") for c in range(2)]
        wk_sb = [const.tile([KT, H], F16, tag=f"wk{c}", name=f"wk{c}") for c in range(2)]
        qT_sb = [
            const.tile([KT, B * QS], F16, tag=f"qt{c}", name=f"qt{c}") for c in range(2)
        ]
        for c in range(2):
            nc.sync.dma_start(wq_sb[c][:], wq_d[c])
            nc.sync.dma_start(wk_sb[c][:], wk_d[c])
            nc.sync.dma_start(qT_sb[c][:], qT_d[c])
        wv_sb = const.tile([H, 1], F16, tag="wv", name="wv_sb")
        nc.sync.dma_start(wv_sb[:], wv_d[:])
        qproj_sb = const.tile([H, B * QS], F32, tag="qproj", name="qproj_sb")

        # transposed scores: partition = key-in-tile, free = task*QS + j
        S = spool.tile([KT, T * QS], F32)
        # partial-tile columns are only written up to their valid rows;
        # zero once so exp() of the stale rows is finite (killed by V=0)
        nc.vector.memset(S[:, :], 0.0)

        # ---- phase 0: query projection ----
        with tc.tile_pool(name="qp_psum", bufs=1, space="PSUM") as qpp:
            qp = qpp.tile([H, B * QS], F32)
            for c in range(2):
                nc.tensor.matmul(
                    qp[:], wq_sb[c][:], qT_sb[c][:], start=(c == 0), stop=(c == 1)
                )
            nc.vector.tensor_copy(qproj_sb[:], qp[:])

        vals_sb = {}  # processed-batch index -> [128, nt*(DV+1)] tile

        with (
            tc.tile_pool(name="kt_sb", bufs=4) as ktp,
            tc.tile_pool(name="kp_psum", bufs=4, space="PSUM") as kpp,
            tc.tile_pool(name="kproj_sb", bufs=3) as kjp,
            tc.tile_pool(name="prebias", bufs=3) as pbp,
            tc.tile_pool(name="feat", bufs=3) as fp,
        ):
            kprojs = {}
            prebiases = {}

            def emit_prebias(i):
                # prebias[:, j*vl2:(j+1)*vl2] = kproj + qproj[:, bq]  (DVE 4x fp16)
                vl_i = nvalid[i]
                vl2 = vl_i + (vl_i & 1)  # even segment width keeps 4B alignment
                kproj = kprojs.pop(i)
                pb = pbp.tile([H, QS * KLEN], F16, tag="pb", name="pb")
                for j in range(QS):
                    bq = order[i] * QS + j
                    nc.vector.tensor_scalar_add(
                        pb[:, j * vl2 : j * vl2 + vl_i],
                        kproj[:, :vl_i],
                        qproj_sb[:, bq : bq + 1],
                    )
                prebiases[i] = pb

            def emit_kproj(i):
                nt = ntiles[i]
                vlpad = nt * KT
                o = int(offs[i])
                # one contiguous DMA for the whole batch's [V|1], then keys^T
                vt = vpool.tile([KT, 8 * (DV + 1)], F16, tag="v1", name="vt")
                nc.sync.dma_start(
                    vt[:, : nt * (DV + 1)],
                    v1_d[:, o * (DV + 1) : (o + nt) * (DV + 1)],
                )
                vals_sb[i] = vt
                kt = ktp.tile([KT, 8 * 2 * KT], F16, tag="kt", name="kt")
                nc.sync.dma_start(
                    kt[:, : nt * 2 * KT], kT_d[:, o * 2 * KT : (o + nt) * 2 * KT]
                )
                vl_i = nvalid[i]
                kproj = kjp.tile([H, KLEN], F16, tag="kproj", name="kproj")
                for t0 in range(0, nt, 4):
                    t1 = min(t0 + 4, nt)
                    cw = min(vl_i, t1 * KT) - t0 * KT
                    kp = kpp.tile([H, 512], F32, tag="kp", name="kp")
                    for t in range(t0, t1):
                        w = min(KT, vl_i - t * KT)
                        sl = kp[:, (t - t0) * KT : (t - t0) * KT + w]
                        k0 = kt[:, (2 * t) * KT : (2 * t) * KT + w]
                        k1 = kt[:, (2 * t + 1) * KT : (2 * t + 1) * KT + w]
                        nc.tensor.matmul(sl, wk_sb[0][:], k0, start=True, stop=False)
                        nc.tensor.matmul(sl, wk_sb[1][:], k1, start=False, stop=True)
                    nc.vector.tensor_copy(
                        kproj[:, t0 * KT : t0 * KT + cw], kp[:, :cw]
                    )
                kprojs[i] = kproj

            def emit_exp_and_av_finetail(i):
                # last batch: per-task exp+AV so the tail chain is short
                nt = ntiles[i]
                b = order[i]
                Ob = opool.tile([QS, DV + 1], F32, tag="ob", name="Ob")
                for t in range(nt):
                    task = int(offs[i]) + t
                    et = epool.tile([KT, QS], F16, tag="et", name="et", bufs=2)
                    nc.scalar.activation(
                        et[:, :],
                        S[:, task * QS : (task + 1) * QS],
                        mybir.ActivationFunctionType.Exp,
                    )
                    nc.tensor.matmul(
                        Ob[:],
                        et[:, :],
                        vals_sb[i][:, t * (DV + 1) : (t + 1) * (DV + 1)],
                        start=(t == 0),
                        stop=(t == nt - 1),
                    )
                recip = fin.tile([QS, 1], F32, tag="recip", name="recip", bufs=2)
                nc.vector.reciprocal(recip[:], Ob[:, DV : DV + 1])
                outf = fin.tile([QS, DV], F32, tag="outf", name="outf", bufs=3)
                nc.vector.tensor_scalar_mul(outf[:], Ob[:, :DV], recip[:])
                nc.sync.dma_start(out_d[b * QS : (b + 1) * QS, :], outf[:])

            def emit_exp_and_av(i0, i1):
                # exp over the column range of processed batches [i0, i1)
                c0, c1 = int(offs[i0]) * QS, int(offs[i1]) * QS
                e = epool.tile([KT, EXP_GROUP * 8 * QS], F16, tag="e", name="e", bufs=3)
                nc.scalar.activation(
                    e[:, : c1 - c0], S[:, c0:c1], mybir.ActivationFunctionType.Exp
                )
                for i in range(i0, i1):
                    b = order[i]
                    nt = ntiles[i]
                    Ob = opool.tile([QS, DV + 1], F32, tag="ob", name="Ob")
                    for t in range(nt):
                        ec = (int(offs[i]) + t) * QS - c0
                        nc.tensor.matmul(
                            Ob[:],
                            e[:, ec : ec + QS],
                            vals_sb[i][:, t * (DV + 1) : (t + 1) * (DV + 1)],
                            start=(t == 0),
                            stop=(t == nt - 1),
                        )
                    vals_sb[i] = None
                    recip = fin.tile([QS, 1], F32, tag="recip", name="recip", bufs=2)
                    nc.vector.reciprocal(recip[:], Ob[:, DV : DV + 1])
                    outf = fin.tile([QS, DV], F32, tag="outf", name="outf", bufs=3)
                    nc.vector.tensor_scalar_mul(outf[:], Ob[:, :DV], recip[:])
                    nc.sync.dma_start(out_d[b * QS : (b + 1) * QS, :], outf[:])

            # kproj two batches ahead, prebias one batch ahead, so PE/DVE
            # feed ACT's single wide tanh per batch without stalls.
            emit_kproj(0)
            emit_kproj(1)
            for i in range(B):
                if i + 1 < B:
                    emit_prebias(i + 1)
                if i + 2 < B:
                    emit_kproj(i + 2)
                nt = ntiles[i]
                vl_i = nvalid[i]
                vl2 = vl_i + (vl_i & 1)
                if i == 0:
                    # ramp: per-q biased tanh straight off kproj — shorter
                    # dependency chain than the prebias path
                    kproj0 = kprojs.pop(0)
                    feat = fp.tile([H, QS * KLEN], F16, tag="feat", name="feat")
                    for j in range(QS):
                        bq = order[0] * QS + j
                        nc.scalar.activation(
                            feat[:, j * vl2 : j * vl2 + vl_i],
                            kproj0[:, :vl_i],
                            mybir.ActivationFunctionType.Tanh,
                            bias=qproj_sb[:, bq : bq + 1],
                        )
                else:
                    pb = prebiases.pop(i)
                    feat = fp.tile([H, QS * KLEN], F16, tag="feat", name="feat")
                    nc.scalar.activation(
                        feat[:, : QS * vl2],
                        pb[:, : QS * vl2],
                        mybir.ActivationFunctionType.Tanh,
                    )
                for j in range(QS):
                    for t in range(nt):
                        w = min(KT, vl_i - t * KT)
                        gcol = (int(offs[i]) + t) * QS + j
                        nc.tensor.matmul(
                            S[:w, gcol : gcol + 1],
                            feat[:, j * vl2 + t * KT : j * vl2 + t * KT + w],
                            wv_sb[:],
                            start=True,
                            stop=True,
                        )
                # lag exp/AV by one group: by the time ACT reaches this exp
                # in its in-order stream, the group's matvecs are long done,
                # so exp doesn't head-of-line-block the next tanh.
                if i % EXP_GROUP == EXP_GROUP - 1 and i >= 2 * EXP_GROUP - 1:
                    emit_exp_and_av(i - 2 * EXP_GROUP + 1, i - EXP_GROUP + 1)
            emit_exp_and_av(B - EXP_GROUP, B - 1)
            emit_exp_and_av_finetail(B - 1)

    nc.compile()
    return nc


def kernel(queries, keys, values, valid_lens, Wq, Wk, wv):
    queries = np.asarray(queries, dtype=np.float32)
    keys = np.asarray(keys, dtype=np.float32)
    values = np.asarray(values, dtype=np.float32)
    vl = np.asarray(valid_lens).astype(np.int64)
    Wq = np.asarray(Wq, dtype=np.float32)
    Wk = np.asarray(Wk, dtype=np.float32)
    wv = np.asarray(wv, dtype=np.float32)

    tiles = np.maximum(1, -(-vl // KT))  # ceil, >=1
    asc = np.argsort(tiles, kind="stable")
    # small batches first (fast ramp, light DMA warmup flood), then
    # descending sizes, with the very smallest batch LAST (shortest tail)
    order = np.concatenate([asc[1:4], asc[4:][::-1], asc[:1]])
    T = int(tiles.sum())

    # packed keys^T, partition-major: column block per task of [2*128] (d-chunks)
    kT_pack = np.empty((KT, T, 2, KT), dtype=np.float16)
    v1_pack = np.empty((KT, T, DV + 1), dtype=np.float16)
    v1_pack[:, :, DV] = 1.0
    i = 0
    for i_proc in range(B):
        b = int(order[i_proc])
        nt = int(tiles[b])
        for t in range(nt):
            ksl = keys[b, t * KT : (t + 1) * KT, :]  # [128 k, 256 d]
            # kT_pack[p, i, c, :] = keys[b, t*KT + (c*128..), p]^T chunks
            kT_pack[:, i, 0, :] = ksl[:, :KT].T
            kT_pack[:, i, 1, :] = ksl[:, KT:].T
            v1_pack[:, i, :DV] = values[b, t * KT : (t + 1) * KT, :]
            if t == nt - 1:
                rows = int(vl[b]) - t * KT
                v1_pack[rows:, i, :] = 0.0  # mask invalid keys via V and ones col
            i += 1
    kT_pack = kT_pack.reshape(KT, T * 2 * KT)
    v1_pack = v1_pack.reshape(KT, T * (DV + 1))

    wqc = Wq.reshape(2, KT, H).astype(np.float16)
    wkc = Wk.reshape(2, KT, H).astype(np.float16)
    wv_c = wv.reshape(H, 1).astype(np.float16)

    nc = _build_graph(order, tiles, vl)

    in_maps = []
    for c in range(N_CORES):
        qc = queries[:, c * QS : (c + 1) * QS, :].reshape(B * QS, D)
        qT = np.ascontiguousarray(qc.T).reshape(2, KT, B * QS).astype(np.float16)
        in_maps.append(
            {
                "kT": kT_pack,
                "v1": v1_pack,
                "qT": qT,
                "wqc": wqc,
                "wkc": wkc,
                "wv": wv_c,
            }
        )

    res = run_bass_kernel_spmd(nc, in_maps, core_ids=list(range(N_CORES)))

    out = np.empty((B, Q, DV), dtype=np.float32)
    for c in range(N_CORES):
        out[:, c * QS : (c + 1) * QS, :] = res.results[c]["out"].reshape(B, QS, DV)
    return out


# revision 43
# speedup vs baseline: 1.1668x; 1.0209x over previous
"""AdditiveAttention (d2l-style) Trainium2 kernel, SPMD over 8 NeuronCores.

Problem shapes: B=16, Q=64, K=1024, DQ=DK=256, H=128, DV=256.

Sharding: data-parallel over the QUERY axis (8 queries per core), with every
core processing all 16 batches. This keeps the single SPMD instruction
stream identical across cores while allowing the graph (built at runtime
from the actual valid_lens values) to skip fully-invalid key tiles — a
large win since valid_lens average ~K/2.

Per-core pipeline (batches processed smallest-first, software-pipelined):
  - host packs keys^T (valid 128-tiles only, partition-major so each batch
    is ONE contiguous DMA) and [values | ones] likewise, both fp16
  - PE: kproj^T = Wk^T @ keys^T per tile; qproj^T likewise
  - ACT: feat = tanh(kproj^T + qproj_col) with qproj as per-partition bias
  - PE: scores col = feat_tile^T @ wv into one [128k, T*8] PSUM tile
        (transposed scores; PE outputs must start at partition 0/32/64)
  - ACT: e = exp(scores) per group of batches (no max-subtract needed;
        |scores| <= sum|wv| ~ 9, exp stays in fp16 range)
  - PE: [out | denom] = e_task^T @ [V | 1] accumulated over valid tiles;
        invalid tail rows of the last tile are zeroed in V on the host
  - DVE: out = out * (1/denom) ; DMA out, all per-batch (no serial tail)
"""

import sys

sys.path.insert(0, "/opt/trn_rl_repo")

from contextlib import ExitStack

import numpy as np

import concourse.bass as bass  # noqa: F401  (bass must import before tile)
import concourse.tile as tile
from concourse import bacc, mybir
from concourse.bass_utils import run_bass_kernel_spmd

B, Q, KLEN, D, H, DV = 16, 64, 1024, 256, 128, 256
N_CORES = 8
QS = Q // N_CORES  # queries per core = 8
KT = 128  # key tile size
EXP_GROUP = 2  # batches per exp instruction

F16 = mybir.dt.float16
F32 = mybir.dt.float32


def _build_graph(order, tiles, vls):
    """order: batch processing order; tiles[b]: valid 128-key tiles; vls[b]: valid_lens."""
    ntiles = [int(tiles[b]) for b in order]
    nvalid = [int(vls[b]) for b in order]
    T = int(sum(ntiles))
    # offs[i] = first task index of the i-th processed batch
    offs = np.concatenate([[0], np.cumsum(ntiles)]).astype(int)

    nc = bacc.Bacc("TRN2", target_bir_lowering=False, debug=False, num_devices=N_CORES)
    # per-batch contiguous packs, partition-major: kT[i] is [128, nt_i*256]
    kT_d = nc.dram_tensor("kT", [KT, T * 2 * KT], F16, kind="ExternalInput").ap()
    v1_d = nc.dram_tensor("v1", [KT, T * (DV + 1)], F16, kind="ExternalInput").ap()
    qT_d = nc.dram_tensor("qT", [2, KT, B * QS], F16, kind="ExternalInput").ap()
    wq_d = nc.dram_tensor("wqc", [2, KT, H], F16, kind="ExternalInput").ap()
    wk_d = nc.dram_tensor("wkc", [2, KT, H], F16, kind="ExternalInput").ap()
    wv_d = nc.dram_tensor("wv", [H, 1], F16, kind="ExternalInput").ap()
    out_d = nc.dram_tensor("out", [B * QS, DV], F32, kind="ExternalOutput").ap()

    with tile.TileContext(nc) as tc, ExitStack() as ctx:
        const = ctx.enter_context(tc.tile_pool(name="const", bufs=1))
        spool = ctx.enter_context(tc.tile_pool(name="s_psum", bufs=1, space="PSUM"))
        epool = ctx.enter_context(tc.tile_pool(name="e_sb", bufs=2))
        vpool = ctx.enter_context(tc.tile_pool(name="vals", bufs=6))
        fin = ctx.enter_context(tc.tile_pool(name="fin", bufs=1))
        opool = ctx.enter_context(tc.tile_pool(name="o_psum", bufs=2, space="PSUM"))

        # constants
        wq_sb = [const.tile([KT, H], F16, tag=f"wq{c}", name=f"wq{c}") for c in range(2)]
        wk_sb = [const.tile([KT, H], F16, tag=f"wk{c}", name=f"wk{c}") for c in range(2)]
        qT_sb = [
            const.tile([KT, B * QS], F16, tag=f"qt{c}", name=f"qt{c}") for c in range(2)
        ]
        for c in range(2):
            nc.sync.dma_start(wq_sb[c][:], wq_d[c])
            nc.sync.dma_start(wk_sb[c][:], wk_d[c])
            nc.sync.dma_start(qT_sb[c][:], qT_d[c])
        wv_sb = const.tile([H, 1], F16, tag="wv", name="wv_sb")
        nc.sync.dma_start(wv_sb[:], wv_d[:])
        qproj_sb = const.tile([H, B * QS], F32, tag="qproj", name="qproj_sb")

        # transposed scores: partition = key-in-tile, free = task*QS + j
        S = spool.tile([KT, T * QS], F32)
        # partial-tile columns are only written up to their valid rows;
        # zero once so exp() of the stale rows is finite (killed by V=0)
        nc.vector.memset(S[:, :], 0.0)

        # ---- phase 0: query projection ----
        with tc.tile_pool(name="qp_psum", bufs=1, space="PSUM") as qpp:
            qp = qpp.tile([H, B * QS], F32)
            for c in range(2):
                nc.tensor.matmul(
                    qp[:], wq_sb[c][:], qT_sb[c][:], start=(c == 0), stop=(c == 1)
                )
            nc.vector.tensor_copy(qproj_sb[:], qp[:])

        vals_sb = {}  # processed-batch index -> [128, nt*(DV+1)] tile

        with (
            tc.tile_pool(name="kt_sb", bufs=4) as ktp,
            tc.tile_pool(name="kp_psum", bufs=4, space="PSUM") as kpp,
            tc.tile_pool(name="kproj_sb", bufs=3) as kjp,
            tc.tile_pool(name="prebias", bufs=3) as pbp,
            tc.tile_pool(name="feat", bufs=4) as fp,
        ):
            kprojs = {}
            prebiases = {}

            def emit_prebias(i):
                # prebias[:, j*vl2:(j+1)*vl2] = kproj + qproj[:, bq]  (DVE 4x fp16)
                vl_i = nvalid[i]
                vl2 = vl_i + (vl_i & 1)  # even segment width keeps 4B alignment
                kproj = kprojs.pop(i)
                pb = pbp.tile([H, QS * KLEN], F16, tag="pb", name="pb")
                for j in range(QS):
                    bq = order[i] * QS + j
                    nc.vector.tensor_scalar_add(
                        pb[:, j * vl2 : j * vl2 + vl_i],
                        kproj[:, :vl_i],
                        qproj_sb[:, bq : bq + 1],
                    )
                prebiases[i] = pb

            def emit_kproj(i):
                nt = ntiles[i]
                vlpad = nt * KT
                o = int(offs[i])
                if i == 0:
                    # ramp batch: per-tile DMA/proj/copy for the shortest
                    # possible chain to the first tanh
                    vt = vpool.tile([KT, 8 * (DV + 1)], F16, tag="v1", name="vt")
                    nc.sync.dma_start(
                        vt[:, : nt * (DV + 1)],
                        v1_d[:, o * (DV + 1) : (o + nt) * (DV + 1)],
                    )
                    vals_sb[i] = vt
                    vl_i = nvalid[i]
                    kproj = kjp.tile([H, KLEN], F16, tag="kproj", name="kproj")
                    for t in range(nt):
                        w = min(KT, vl_i - t * KT)
                        kt = ktp.tile([KT, 8 * 2 * KT], F16, tag="kt", name="kt")
                        nc.sync.dma_start(
                            kt[:, : 2 * KT],
                            kT_d[:, (o + t) * 2 * KT : (o + t + 1) * 2 * KT],
                        )
                        kp = kpp.tile([H, 512], F32, tag="kp", name="kp")
                        nc.tensor.matmul(
                            kp[:, :w], wk_sb[0][:], kt[:, :w], start=True, stop=False
                        )
                        nc.tensor.matmul(
                            kp[:, :w], wk_sb[1][:], kt[:, KT : KT + w],
                            start=False, stop=True,
                        )
                        nc.vector.tensor_copy(
                            kproj[:, t * KT : t * KT + w], kp[:, :w]
                        )
                    kprojs[i] = kproj
                    return

                # keys^T for the whole batch in one DMA ([V|1] is emitted
                # later, from the main loop, to keep the warmup flood light)
                kt = ktp.tile([KT, 8 * 2 * KT], F16, tag="kt", name="kt")
                nc.sync.dma_start(
                    kt[:, : nt * 2 * KT], kT_d[:, o * 2 * KT : (o + nt) * 2 * KT]
                )
                vl_i = nvalid[i]
                kproj = kjp.tile([H, KLEN], F16, tag="kproj", name="kproj")
                for t0 in range(0, nt, 4):
                    t1 = min(t0 + 4, nt)
                    cw = min(vl_i, t1 * KT) - t0 * KT
                    kp = kpp.tile([H, 512], F32, tag="kp", name="kp")
                    for t in range(t0, t1):
                        w = min(KT, vl_i - t * KT)
                        sl = kp[:, (t - t0) * KT : (t - t0) * KT + w]
                        k0 = kt[:, (2 * t) * KT : (2 * t) * KT + w]
                        k1 = kt[:, (2 * t + 1) * KT : (2 * t + 1) * KT + w]
                        nc.tensor.matmul(sl, wk_sb[0][:], k0, start=True, stop=False)
                        nc.tensor.matmul(sl, wk_sb[1][:], k1, start=False, stop=True)
                    nc.vector.tensor_copy(
                        kproj[:, t0 * KT : t0 * KT + cw], kp[:, :cw]
                    )
                kprojs[i] = kproj

            def emit_exp_and_av_finetail(i):
                # last batch: per-task exp+AV so the tail chain is short
                nt = ntiles[i]
                b = order[i]
                Ob = opool.tile([QS, DV + 1], F32, tag="ob", name="Ob")
                for t in range(nt):
                    task = int(offs[i]) + t
                    et = epool.tile([KT, QS], F16, tag="et", name="et", bufs=2)
                    nc.scalar.activation(
                        et[:, :],
                        S[:, task * QS : (task + 1) * QS],
                        mybir.ActivationFunctionType.Exp,
                    )
                    nc.tensor.matmul(
                        Ob[:],
                        et[:, :],
                        vals_sb[i][:, t * (DV + 1) : (t + 1) * (DV + 1)],
                        start=(t == 0),
                        stop=(t == nt - 1),
                    )
                recip = fin.tile([QS, 1], F32, tag="recip", name="recip", bufs=2)
                nc.vector.reciprocal(recip[:], Ob[:, DV : DV + 1])
                outf = fin.tile([QS, DV], F32, tag="outf", name="outf", bufs=3)
                nc.vector.tensor_scalar_mul(outf[:], Ob[:, :DV], recip[:])
                nc.sync.dma_start(out_d[b * QS : (b + 1) * QS, :], outf[:])

            def emit_exp_and_av(i0, i1):
                # exp over the column range of processed batches [i0, i1)
                c0, c1 = int(offs[i0]) * QS, int(offs[i1]) * QS
                e = epool.tile([KT, EXP_GROUP * 8 * QS], F16, tag="e", name="e", bufs=3)
                nc.scalar.activation(
                    e[:, : c1 - c0], S[:, c0:c1], mybir.ActivationFunctionType.Exp
                )
                for i in range(i0, i1):
                    b = order[i]
                    nt = ntiles[i]
                    Ob = opool.tile([QS, DV + 1], F32, tag="ob", name="Ob")
                    for t in range(nt):
                        ec = (int(offs[i]) + t) * QS - c0
                        nc.tensor.matmul(
                            Ob[:],
                            e[:, ec : ec + QS],
                            vals_sb[i][:, t * (DV + 1) : (t + 1) * (DV + 1)],
                            start=(t == 0),
                            stop=(t == nt - 1),
                        )
                    vals_sb[i] = None
                    recip = fin.tile([QS, 1], F32, tag="recip", name="recip", bufs=2)
                    nc.vector.reciprocal(recip[:], Ob[:, DV : DV + 1])
                    outf = fin.tile([QS, DV], F32, tag="outf", name="outf", bufs=3)
                    nc.vector.tensor_scalar_mul(outf[:], Ob[:, :DV], recip[:])
                    nc.sync.dma_start(out_d[b * QS : (b + 1) * QS, :], outf[:])

            # kproj two batches ahead, prebias one batch ahead, so PE/DVE
            # feed ACT's single wide tanh per batch without stalls.
            emit_kproj(0)
            emit_kproj(1)
            for i in range(B):
                if i + 1 < B:
                    emit_prebias(i + 1)
                if i + 2 < B:
                    emit_kproj(i + 2)
                nt = ntiles[i]
                vl_i = nvalid[i]
                vl2 = vl_i + (vl_i & 1)
                if i > 0:
                    o = int(offs[i])
                    vt = vpool.tile([KT, 8 * (DV + 1)], F16, tag="v1", name="vt")
                    nc.sync.dma_start(
                        vt[:, : nt * (DV + 1)],
                        v1_d[:, o * (DV + 1) : (o + nt) * (DV + 1)],
                    )
                    vals_sb[i] = vt
                if i == 0:
                    # ramp: per-(tile, q) biased tanh straight off kproj —
                    # each tile's tanh gates only on that tile's projection
                    kproj0 = kprojs.pop(0)
                    feat = fp.tile([H, QS * KLEN], F16, tag="feat", name="feat")
                    for t in range(nt):
                        w = min(KT, vl_i - t * KT)
                        for j in range(QS):
                            bq = order[0] * QS + j
                            nc.scalar.activation(
                                feat[:, j * vl2 + t * KT : j * vl2 + t * KT + w],
                                kproj0[:, t * KT : t * KT + w],
                                mybir.ActivationFunctionType.Tanh,
                                bias=qproj_sb[:, bq : bq + 1],
                            )
                            gcol = (int(offs[0]) + t) * QS + j
                            nc.tensor.matmul(
                                S[:w, gcol : gcol + 1],
                                feat[:, j * vl2 + t * KT : j * vl2 + t * KT + w],
                                wv_sb[:],
                                start=True,
                                stop=True,
                            )
                else:
                    pb = prebiases.pop(i)
                    feat = fp.tile([H, QS * KLEN], F16, tag="feat", name="feat")
                    nc.scalar.activation(
                        feat[:, : QS * vl2],
                        pb[:, : QS * vl2],
                        mybir.ActivationFunctionType.Tanh,
                    )
                    for j in range(QS):
                        for t in range(nt):
                            w = min(KT, vl_i - t * KT)
                            gcol = (int(offs[i]) + t) * QS + j
                            nc.tensor.matmul(
                                S[:w, gcol : gcol + 1],
                                feat[:, j * vl2 + t * KT : j * vl2 + t * KT + w],
                                wv_sb[:],
                                start=True,
                                stop=True,
                            )
                # lag exp/AV by one group: by the time ACT reaches this exp
                # in its in-order stream, the group's matvecs are long done,
                # so exp doesn't head-of-line-block the next tanh.
                if (
                    i % EXP_GROUP == EXP_GROUP - 1
                    and i >= 2 * EXP_GROUP - 1
                    and i - EXP_GROUP + 1 <= B - 3
                ):
                    emit_exp_and_av(i - 2 * EXP_GROUP + 1, i - EXP_GROUP + 1)
            # tail: per-task exp+AV for the last three batches so their
            # chains collapse right behind the final tanh/matvecs
            emit_exp_and_av(B - 4, B - 3)
            for i in (B - 3, B - 2, B - 1):
                emit_exp_and_av_finetail(i)

    nc.compile()
    return nc


def kernel(queries, keys, values, valid_lens, Wq, Wk, wv):
    queries = np.asarray(queries, dtype=np.float32)
    keys = np.asarray(keys, dtype=np.float32)
    values = np.asarray(values, dtype=np.float32)
    vl = np.asarray(valid_lens).astype(np.int64)
    Wq = np.asarray(Wq, dtype=np.float32)
    Wk = np.asarray(Wk, dtype=np.float32)
    wv = np.asarray(wv, dtype=np.float32)

    tiles = np.maximum(1, -(-vl // KT))  # ceil, >=1
    asc = np.lexsort((vl, tiles))
    # small batches first (fast ramp, light DMA warmup flood), then
    # descending sizes, with the two smallest batches LAST (shortest tail)
    order = np.concatenate([asc[2:4], asc[4:][::-1], asc[1::-1]])
    T = int(tiles.sum())

    # packed keys^T, partition-major: column block per task of [2*128] (d-chunks)
    kT_pack = np.empty((KT, T, 2, KT), dtype=np.float16)
    v1_pack = np.empty((KT, T, DV + 1), dtype=np.float16)
    v1_pack[:, :, DV] = 1.0
    i = 0
    for i_proc in range(B):
        b = int(order[i_proc])
        nt = int(tiles[b])
        for t in range(nt):
            ksl = keys[b, t * KT : (t + 1) * KT, :]  # [128 k, 256 d]
            # kT_pack[p, i, c, :] = keys[b, t*KT + (c*128..), p]^T chunks
            kT_pack[:, i, 0, :] = ksl[:, :KT].T
            kT_pack[:, i, 1, :] = ksl[:, KT:].T
            v1_pack[:, i, :DV] = values[b, t * KT : (t + 1) * KT, :]
            if t == nt - 1:
                rows = int(vl[b]) - t * KT
                v1_pack[rows:, i, :] = 0.0  # mask invalid keys via V and ones col
            i += 1
    kT_pack = kT_pack.reshape(KT, T * 2 * KT)
    v1_pack = v1_pack.reshape(KT, T * (DV + 1))

    wqc = Wq.reshape(2, KT, H).astype(np.float16)
    wkc = Wk.reshape(2, KT, H).astype(np.float16)
    wv_c = wv.reshape(H, 1).astype(np.float16)

    nc = _build_graph(order, tiles, vl)

    in_maps = []
    for c in range(N_CORES):
        qc = queries[:, c * QS : (c + 1) * QS, :].reshape(B * QS, D)
        qT = np.ascontiguousarray(qc.T).reshape(2, KT, B * QS).astype(np.float16)
        in_maps.append(
            {
                "kT": kT_pack,
                "v1": v1_pack,
                "qT": qT,
                "wqc": wqc,
                "wkc": wkc,
                "wv": wv_c,
            }
        )

    res = run_bass_kernel_spmd(nc, in_maps, core_ids=list(range(N_CORES)))

    out = np.empty((B, Q, DV), dtype=np.float32)
    for c in range(N_CORES):
        out[:, c * QS : (c + 1) * QS, :] = res.results[c]["out"].reshape(B, QS, DV)
    return out
